# revision 1
# baseline (speedup 1.0000x reference)
"""Distributed Trainium2 kernel for nn_ContrastiveLoss (SimCLR InfoNCE loss).

Math (matches the JAX reference):
    cos = (z/||z||) @ (z/||z||)^T          # [N, N], N=8192, D=1024
    logits = cos / T  (T=0.1), diag masked to -inf (exp -> 0)
    nll_i = -logits[i, (i+N/2) mod N] + log(sum_j exp(logits[i, j]))
    out = mean(nll)

Sharding: rows of z are split across 8 NeuronCores (1024 rows each).
Each core gets a column-ROLLED copy of normalized z^T (zr_c = zhat.T
rolled left by c*1024 columns) so that the self-diagonal block and the
positive-pair block land at *core-independent* static column offsets —
all 8 cores run the identical program on different data.

Per core: zhat_slab^T @ zhat^T_rolled via 1024 accumulating float32r
matmuls (PE, fp22 precision, full rate), fused exp+row-sum on the
Scalar engine (accum_out), diagonal extraction via identity-mask
tensor_tensor_reduce on the Vector engine. Row nll vectors are DMA'd
out; the host computes the final mean.
"""

import numpy as np

N, D = 8192, 1024
NCORES = 8
ROWS = N // NCORES      # 1024 rows per core
MT = ROWS // 128        # 8 m-tiles of 128 rows
KT = D // 128           # 8 k-tiles (contraction)
NTILE = 512             # moving-dim tile (one PSUM bank of fp32)
NT = N // NTILE         # 16 n-tiles
TEMP_INV = 10.0         # 1/temperature


def _import_concourse():
    import sys
    try:
        import concourse.bass  # noqa: F401
    except ImportError:
        for p in ("/root/.axon_site/_ro/trn_rl_repo", "/opt/trn_rl_repo"):
            if p not in sys.path:
                sys.path.insert(0, p)
        import concourse.bass  # noqa: F401


def build_program():
    _import_concourse()
    import concourse.mybir as mybir
    import concourse.tile as tile
    from concourse import bacc
    from concourse.masks import make_identity

    f32 = mybir.dt.float32
    f32r = mybir.dt.float32r
    Act = mybir.ActivationFunctionType
    Alu = mybir.AluOpType

    nc = bacc.Bacc()
    zr = nc.declare_dram_parameter("zr", [D, N], f32r, isOutput=False)
    out = nc.declare_dram_parameter("out", [128, MT], f32, isOutput=True)

    # zr[d, j]: d = k*128 + p (partition p, k-tile k); j = global free col
    zr_pkn = zr.rearrange("(k p) n -> p k n", p=128)
    zr_lhs = zr.rearrange("(k p) (m f) -> p k m f", p=128, f=128)

    with tile.TileContext(nc) as tc:
        with (
            tc.tile_pool(name="consts", bufs=1) as consts,
            tc.tile_pool(name="lhsp", bufs=1) as lhsp,
            tc.tile_pool(name="rhsp", bufs=3) as rhsp,
            tc.tile_pool(name="psump", bufs=8, space="PSUM") as psump,
            tc.tile_pool(name="escp", bufs=3) as escp,
            tc.tile_pool(name="smallp", bufs=4) as smallp,
            tc.tile_pool(name="accp", bufs=1) as accp,
        ):
            ident = consts.tile([128, 128], f32)
            make_identity(nc, ident)
            # DVE warmup read of ident: advances DVE's observed GpSimd
            # vector-clock so later tensor_mul(psum, ident) ops carry only
            # one sync wait (walrus: DVE TensorTensor allows a single wait).
            identw = consts.tile([128, 1], f32)
            nc.vector.reduce_max(
                out=identw, in_=ident, axis=mybir.AxisListType.X
            )

            # resident lhsT slab: zr[:, :1024] as [p, k, m, f].
            # Split per (k, m) so the first matmuls only wait on the
            # slices they read, not the whole 4 MB slab.
            lhs_t = lhsp.tile([128, KT, MT, 128], f32r)

            acc = accp.tile([128, MT * NT], f32)   # per-(m, n) exp row sums
            dcol = accp.tile([128, MT], f32)       # self-diag cos values
            pcol = accp.tile([128, MT], f32)       # 10 * positive-pair cos

            for n in range(NT):
                rhs_t = rhsp.tile([128, KT, NTILE], f32r)
                for k in range(KT):
                    nc.sync.dma_start(
                        out=rhs_t[:, k],
                        in_=zr_pkn[:, k, n * NTILE : (n + 1) * NTILE],
                    )
                if n == 0:
                    # lhs DMAs issued after rhs tile 0 so matmul (0,0,0)
                    # starts as soon as ~2.5 MB (not 6 MB) has landed.
                    for m in range(MT):
                        for k in range(KT):
                            nc.sync.dma_start(
                                out=lhs_t[:, k, m], in_=zr_lhs[:, k, m]
                            )
                for m in range(MT):
                    ps = psump.tile([128, NTILE], f32)
                    for k in range(KT):
                        nc.tensor.matmul(
                            ps,
                            lhsT=lhs_t[:, k, m, :],
                            rhs=rhs_t[:, k, :],
                            start=(k == 0),
                            stop=(k == KT - 1),
                        )
                    if n == m // 4:
                        # self-diagonal block: local col m*128 + p
                        off = (m % 4) * 128
                        dtmp = smallp.tile([128, 128], f32, tag="blk")
                        nc.vector.tensor_mul(
                            out=dtmp, in0=ps[:, off : off + 128], in1=ident
                        )
                        nc.vector.reduce_sum(
                            out=dcol[:, m : m + 1], in_=dtmp,
                            axis=mybir.AxisListType.X,
                        )
                    if n == MT + m // 4:
                        # positive-pair block: local col 4096 + m*128 + p
                        off = (m % 4) * 128
                        ptmp = smallp.tile([128, 128], f32, tag="blk")
                        nc.vector.tensor_mul(
                            out=ptmp, in0=ps[:, off : off + 128], in1=ident
                        )
                        nc.vector.reduce_sum(
                            out=pcol[:, m : m + 1], in_=ptmp,
                            axis=mybir.AxisListType.X,
                        )
                    # exp(10 * cos) with fused row-sum into acc[:, m*NT+n]
                    esc = escp.tile([128, NTILE], f32)
                    idx = m * NT + n
                    nc.scalar.activation(
                        out=esc,
                        in_=ps,
                        func=Act.Exp,
                        scale=TEMP_INV,
                        accum_out=acc[:, idx : idx + 1],
                    )

            # Batched epilogue: one op per stage over all 8 m-tiles, so
            # Exp and Ln each run once (no ACT table-set thrashing).
            outt = accp.tile([128, MT], f32)
            sall = accp.tile([128, MT], f32)
            nc.vector.reduce_sum(
                out=sall,
                in_=acc.rearrange("p (m n) -> p m n", m=MT),
                axis=mybir.AxisListType.X,
            )
            edall = accp.tile([128, MT], f32)
            nc.scalar.activation(
                out=edall, in_=dcol, func=Act.Exp, scale=TEMP_INV
            )
            s2all = accp.tile([128, MT], f32)
            nc.vector.tensor_sub(out=s2all, in0=sall, in1=edall)
            lseall = accp.tile([128, MT], f32)
            nc.scalar.activation(out=lseall, in_=s2all, func=Act.Ln)
            # nll = lse - 10 * pos_cos  (pcol holds raw cos values)
            nc.vector.tensor_scalar_mul(out=outt, in0=pcol, scalar1=-TEMP_INV)
            nc.vector.tensor_add(out=outt, in0=outt, in1=lseall)
            nc.sync.dma_start(out=out[:, :], in_=outt)
    nc.finalize()
    return nc


def make_in_maps(z: np.ndarray) -> list[dict]:
    z = np.ascontiguousarray(np.asarray(z, dtype=np.float32))
    norms = np.sqrt((z.astype(np.float64) ** 2).sum(axis=-1))
    zn = (z / norms[:, None]).astype(np.float32)
    zt = np.ascontiguousarray(zn.T)  # [D, N]
    in_maps = []
    for c in range(NCORES):
        s = c * ROWS
        if s == 0:
            zr = zt
        else:
            zr = np.ascontiguousarray(
                np.concatenate([zt[:, s:], zt[:, :s]], axis=1)
            )
        in_maps.append({"zr": zr})
    return in_maps


def assemble(results: list[dict]) -> np.ndarray:
    # results[c]["out"][p, m] = nll of global row c*1024 + m*128 + p
    nll = np.stack([np.asarray(r["out"], np.float32) for r in results])  # [c,p,m]
    nll = nll.transpose(0, 2, 1).reshape(-1)  # global row order
    return np.float32(nll.mean())


def kernel(z: np.ndarray) -> np.ndarray:
    _import_concourse()
    from concourse.bass_utils import run_bass_kernel_spmd

    nc = build_program()
    in_maps = make_in_maps(z)
    res = run_bass_kernel_spmd(nc, in_maps, core_ids=list(range(NCORES)))
    return assemble(res.results)



# revision 3
# speedup vs baseline: 4.7664x; 4.7664x over previous
"""Distributed Trainium2 kernel for nn_ContrastiveLoss (SimCLR InfoNCE loss).

Math (matches the JAX reference):
    cos = (z/||z||) @ (z/||z||)^T          # [N, N], N=8192, D=1024
    logits = cos / T  (T=0.1), diag masked to -inf (exp -> 0)
    nll_i = -logits[i, (i+N/2) mod N] + log(sum_j exp(logits[i, j]))
    out = mean(nll)

Sharding: rows of z are split across 8 NeuronCores (1024 rows each).
Each core gets a column-ROLLED copy of normalized z^T (zr_c = zhat.T
rolled left by c*1024 columns) so that the self-diagonal block and the
positive-pair block land at *core-independent* static column offsets —
all 8 cores run the identical program on different data.

Inputs are quantized to fp8 e4m3 (scale 16) on the host; the matmuls
run in MatmulPerfMode.DoubleRow (two 128-row contraction slices per
instruction, 2x fp8 rate). The whole 8 MB fp8 slab is SBUF-resident so
lhsT and rhs both slice from it with zero re-reads. Fused exp+row-sum
on the Scalar engine over [128, 2048] PSUM spans (4 banks), ping-ponged
against the PE filling the other 4 banks. Diagonal / positive-pair
extraction via identity-mask tensor ops on the Vector engine. Row nll
vectors are DMA'd out; the host computes the final mean.
"""

import numpy as np

N, D = 8192, 1024
NCORES = 8
ROWS = N // NCORES      # 1024 rows per core
MT = ROWS // 128        # 8 m-tiles of 128 rows
KT = D // 128           # 8 k-tiles (contraction)
KP = KT // 2            # 4 DoubleRow k-pairs
NTILE = 512             # PSUM bank of fp32
HTILE = 2048            # ACT processing span (4 banks)
NH = N // HTILE         # 4 h-tiles
NPH = HTILE // NTILE    # 4 n-subtiles per h
FP8_SCALE = 16.0        # host quantization scale for e4m3
ACT_SCALE = 10.0 / (FP8_SCALE * FP8_SCALE)  # logits = dots_q * ACT_SCALE


def _import_concourse():
    import sys
    try:
        import concourse.bass  # noqa: F401
    except ImportError:
        for p in ("/root/.axon_site/_ro/trn_rl_repo", "/opt/trn_rl_repo"):
            if p not in sys.path:
                sys.path.insert(0, p)
        import concourse.bass  # noqa: F401


def build_program():
    _import_concourse()
    import concourse.mybir as mybir
    import concourse.tile as tile
    from concourse import bacc
    from concourse.masks import make_identity

    f32 = mybir.dt.float32
    bf16 = mybir.dt.bfloat16
    fp8 = mybir.dt.float8e4
    Act = mybir.ActivationFunctionType
    DR = mybir.MatmulPerfMode.DoubleRow

    nc = bacc.Bacc()
    zr = nc.declare_dram_parameter("zr", [D, N], fp8, isOutput=False)
    out = nc.declare_dram_parameter("out", [128, MT], f32, isOutput=True)

    # zr[d, j]: d = k*128 + p (partition p, k-tile k); j = global free col
    zr_pkn = zr.rearrange("(k p) n -> p k n", p=128)

    with tile.TileContext(nc) as tc:
        with (
            tc.tile_pool(name="consts", bufs=1) as consts,
            tc.tile_pool(name="zsp", bufs=1) as zsp,
            tc.tile_pool(name="psump", bufs=2, space="PSUM") as psump,
            tc.tile_pool(name="escp", bufs=2) as escp,
            tc.tile_pool(name="smallp", bufs=4) as smallp,
            tc.tile_pool(name="accp", bufs=1) as accp,
        ):
            ident = consts.tile([128, 128], f32)
            make_identity(nc, ident)
            # DVE warmup read of ident: advances DVE's observed GpSimd
            # vector-clock so later tensor_mul(psum, ident) ops carry only
            # one sync wait (walrus: DVE TensorTensor allows a single wait).
            identw = consts.tile([128, 1], f32)
            nc.vector.reduce_max(
                out=identw, in_=ident, axis=mybir.AxisListType.X
            )

            # whole fp8 slab resident: [p, k-tile, global col]. lhsT and
            # rhs for every matmul slice straight out of this.
            zs = zsp.tile([128, KT, N], fp8)

            acc = accp.tile([128, MT * NH], f32)    # per-(m, h) exp row sums
            dcol = accp.tile([128, MT], f32)        # self-diag dot_q values
            pcol = accp.tile([128, MT], f32)        # positive-pair dot_q

            # DMA in h-major order so compute on h=0 starts after ~2 MB.
            for h in range(NH):
                for k in range(KT):
                    nc.sync.dma_start(
                        out=zs[:, k, h * HTILE : (h + 1) * HTILE],
                        in_=zr_pkn[:, k, h * HTILE : (h + 1) * HTILE],
                    )

            for h in range(NH):
                for m in range(MT):
                    ps = psump.tile([128, HTILE], f32)
                    for j in range(NPH):
                        n = h * NPH + j
                        for kp in range(KP):
                            nc.tensor.matmul(
                                ps[:, j * NTILE : (j + 1) * NTILE],
                                lhsT=zs[:, 2 * kp : 2 * kp + 2,
                                        m * 128 : (m + 1) * 128],
                                rhs=zs[:, 2 * kp : 2 * kp + 2,
                                       n * NTILE : (n + 1) * NTILE],
                                start=(kp == 0),
                                stop=(kp == KP - 1),
                                perf_mode=DR,
                            )
                    # self-diagonal block: local col m*128 + p -> n-tile
                    # m//4; positive-pair block: col 4096 + m*128 + p ->
                    # n-tile 8 + m//4.  n-tile t lives in h = t//NPH at
                    # sub-offset (t%NPH)*NTILE.
                    for tgt, t in ((dcol, m // 4), (pcol, MT + m // 4)):
                        if h == t // NPH:
                            off = (t % NPH) * NTILE + (m % 4) * 128
                            tmp = smallp.tile([128, 128], f32, tag="blk")
                            nc.vector.tensor_mul(
                                out=tmp, in0=ps[:, off : off + 128], in1=ident
                            )
                            nc.vector.reduce_sum(
                                out=tgt[:, m : m + 1],
                                in_=tmp,
                                axis=mybir.AxisListType.X,
                            )
                    # exp(ACT_SCALE * dots) + fused row-sum into acc
                    esc = escp.tile([128, HTILE], bf16)
                    idx = m * NH + h
                    nc.scalar.activation(
                        out=esc,
                        in_=ps,
                        func=Act.Exp,
                        scale=ACT_SCALE,
                        accum_out=acc[:, idx : idx + 1],
                    )

            # Batched epilogue: one op per stage over all 8 m-tiles, so
            # Exp and Ln each run once (no ACT table-set thrashing).
            outt = accp.tile([128, MT], f32)
            sall = accp.tile([128, MT], f32)
            nc.vector.reduce_sum(
                out=sall,
                in_=acc.rearrange("p (m h) -> p m h", m=MT),
                axis=mybir.AxisListType.X,
            )
            edall = accp.tile([128, MT], f32)
            nc.scalar.activation(
                out=edall, in_=dcol, func=Act.Exp, scale=ACT_SCALE
            )
            s2all = accp.tile([128, MT], f32)
            nc.vector.tensor_sub(out=s2all, in0=sall, in1=edall)
            lseall = accp.tile([128, MT], f32)
            nc.scalar.activation(out=lseall, in_=s2all, func=Act.Ln)
            # nll = lse - ACT_SCALE * pos_dot  (pcol holds raw dot_q)
            nc.vector.tensor_scalar_mul(
                out=outt, in0=pcol, scalar1=-ACT_SCALE
            )
            nc.vector.tensor_add(out=outt, in0=outt, in1=lseall)
            nc.sync.dma_start(out=out[:, :], in_=outt)
    nc.finalize()
    return nc


def make_in_maps(z: np.ndarray) -> list[dict]:
    import ml_dtypes

    z = np.ascontiguousarray(np.asarray(z, dtype=np.float32))
    norms = np.sqrt((z.astype(np.float64) ** 2).sum(axis=-1))
    zn = (z / norms[:, None] * FP8_SCALE).astype(ml_dtypes.float8_e4m3)
    zt = np.ascontiguousarray(zn.T)  # [D, N] fp8
    in_maps = []
    for c in range(NCORES):
        s = c * ROWS
        if s == 0:
            zr = zt
        else:
            zr = np.ascontiguousarray(
                np.concatenate([zt[:, s:], zt[:, :s]], axis=1)
            )
        in_maps.append({"zr": zr})
    return in_maps


def assemble(results: list[dict]) -> np.ndarray:
    # results[c]["out"][p, m] = nll of global row c*1024 + m*128 + p
    nll = np.stack([np.asarray(r["out"], np.float32) for r in results])  # [c,p,m]
    nll = nll.transpose(0, 2, 1).reshape(-1)  # global row order
    return np.float32(nll.mean())


def kernel(z: np.ndarray) -> np.ndarray:
    _import_concourse()
    from concourse.bass_utils import run_bass_kernel_spmd

    nc = build_program()
    in_maps = make_in_maps(z)
    res = run_bass_kernel_spmd(nc, in_maps, core_ids=list(range(NCORES)))
    return assemble(res.results)
